# revision 13
# baseline (speedup 1.0000x reference)
"""Multi-head attention forward (B=2, T=2048, C=1024, 16 heads of dim 64)
sharded 8-way tensor-parallel over heads across 8 TRN2 NeuronCores.

Each core computes 2 heads end-to-end; host sums the 8 partial output
projections (the tensor-parallel all-reduce).

Per-core pipeline (ScalarE exp is the bottleneck: 128 ACTIVATEs of
[128,1024] ~= 147us; everything else is emission-interleaved at kt
granularity so the PE/DVE/DMA work hides under the exp stream):

  k^T/q^T/v^T = w_m^T @ x^T      (per-chunk passes, m-outer, PSUM-chained)
  S^T tile    = 2x2 tile_position-packed matmuls (2 heads x 2 token
                halves run on disjoint 64x64 PE quadrants)
  P^T = exp(S^T)                 (ScalarE, [128, 2, 512] per kt)
  y^T_h = [v_h | 1]^T @ P^T_h    (ones column yields softmax denoms)
  1/d on VectorE (reciprocal_approx_fast), broadcast on GpSimd,
  normalize on VectorE, then partial out-proj on PE.

Matmuls in bf16; softmax statistics and normalization in fp32.
"""

import numpy as np
import ml_dtypes
from contextlib import ExitStack
from collections import deque

import concourse.bass as bass
import concourse.bacc as bacc
import concourse.mybir as mybir
import concourse.tile as tile
from concourse.bass_utils import run_bass_kernel_spmd

F32 = mybir.dt.float32
BF16 = mybir.dt.bfloat16
AFT = mybir.ActivationFunctionType

P = 128
NB = 2        # batches
TB = 2048     # tokens per batch
NT = NB * TB  # 4096 tokens total
C = 1024
KC = C // P   # 8 contraction tiles for the qkv projection
QCH = 512     # q-token chunk
NQC = TB // QCH   # 4 q chunks per batch
NCH = NT // QCH   # 8 chunks total
NKT = TB // P     # 16 k tiles per batch
N_CORES = 8
HD = 64       # head dim
VW = 80       # padded per-head stride in v_aug (32B-aligned for DMA xpose)


def _build_program(nc: bass.Bass):
    xT = nc.declare_dram_parameter("xT", [C, NT], BF16, isOutput=False)[:]
    wqkv = nc.declare_dram_parameter("wqkv", [C, 384], BF16, isOutput=False)[:]
    wproj = nc.declare_dram_parameter("wproj", [2, HD, C], BF16, isOutput=False)[:]
    out = nc.declare_dram_parameter("out", [NT, C], F32, isOutput=True)[:]

    with tile.TileContext(nc) as tc, ExitStack() as ctx:
        singles = ctx.enter_context(tc.tile_pool(name="singles", bufs=1))
        ptp = ctx.enter_context(tc.tile_pool(name="ptp", bufs=2))
        vtp = ctx.enter_context(tc.tile_pool(name="vtp", bufs=2))
        ybp = ctx.enter_context(tc.tile_pool(name="ybp", bufs=2))
        rfp = ctx.enter_context(tc.tile_pool(name="rfp", bufs=2))
        recp = ctx.enter_context(tc.tile_pool(name="recp", bufs=2))
        osb = ctx.enter_context(tc.tile_pool(name="osb", bufs=4))
        psA = ctx.enter_context(tc.tile_pool(name="psA", bufs=2, space="PSUM"))
        psPV = ctx.enter_context(tc.tile_pool(name="psPV", bufs=1, space="PSUM"))
        psM = ctx.enter_context(tc.tile_pool(name="psM", bufs=2, space="PSUM"))

        # ---------------- persistent tensors ----------------
        w_sb = singles.tile([P, KC, 384], BF16, tag="w_sb")
        nc.sync.dma_start(out=w_sb[:], in_=wqkv.rearrange("(kc p) m -> p kc m", p=P))

        wp_sb = singles.tile([P, C], BF16, tag="wp")
        for h in range(2):
            nc.sync.dma_start(out=wp_sb[h * HD : (h + 1) * HD, :], in_=wproj[h])

        xin = singles.tile([P, KC, NT], BF16, tag="xin")
        for cch in range(NCH):
            tsl = slice(cch * QCH, (cch + 1) * QCH)
            nc.sync.dma_start(
                out=xin[:, :, tsl],
                in_=xT[:, tsl].rearrange("(kc p) t -> p kc t", p=P),
            )

        q_sb = singles.tile([P, NT], BF16, tag="q_sb")
        k_sb = singles.tile([P, NT], BF16, tag="k_sb")
        # v_aug[:, i, h, 0:64] = v_h for token tile i; col 64 = ones
        v_aug = singles.tile([P, NT // P, 2, VW], BF16, tag="v_aug")
        nc.vector.memset(v_aug[:, :, :, HD : HD + 1], 1.0)

        # pT tiles are large; keep 2 in flight (PV(j) runs during scores(j+1))
        pT = []
        for _pi in range(2):
            pt_buf = ptp.tile([P, NKT, 2, QCH], BF16, tag="pT", name=f"pT{_pi}")
            pT.append(pt_buf)

        # ---------------- background step queue ----------------
        # Each step costs ~cost x 216ns of PE time; the scores loop pumps
        # ~ALLOW units per kt so background work stays under the ACT-paced
        # exp stream without starving it.
        bg = deque()
        credit = [0.0]
        done = set()  # (m, cch) of completed qkv passes

        def pump(allow):
            credit[0] += allow
            while bg and bg[0][0] <= credit[0]:
                cost, fn = bg.popleft()
                fn()
                credit[0] -= cost

        def need(m, cch):
            # force-emit queued steps until pass (m, cch) has been emitted;
            # emission order IS program order for Tile's dependency tracking
            while (m, cch) not in done:
                assert bg, f"dependency ({m},{cch}) never queued"
                cost, fn = bg.popleft()
                fn()
                credit[0] -= cost

        # ---------------- qkv passes (m in {q:0, k:1, v:2}) -------------
        def emit_half_pass(ps, m, cch, half):
            tsl = slice(cch * QCH, (cch + 1) * QCH)
            for kc in range(half * 4, half * 4 + 4):
                nc.tensor.matmul(
                    ps[:],
                    lhsT=w_sb[:, kc, m * P : (m + 1) * P],
                    rhs=xin[:, kc, tsl],
                    start=(kc == 0),
                    stop=(kc == KC - 1),
                )

        def emit_kq_fin(ps, dst, cch):
            tsl = slice(cch * QCH, (cch + 1) * QCH)
            nc.vector.tensor_copy(out=dst[:, tsl], in_=ps[:])

        def emit_v_fin(ps, cch):
            vt = vtp.tile([P, QCH], BF16, tag="vt")
            nc.vector.tensor_copy(out=vt[:], in_=ps[:])
            for j in range(QCH // P):
                i = cch * (QCH // P) + j
                for h in range(2):
                    nc.sync.dma_start(
                        out=v_aug[:, i, h, 0:HD],
                        in_=vt[h * HD : (h + 1) * HD, j * P : (j + 1) * P],
                        transpose=True,
                    )

        def queue_pass(m, cch):
            # closure-shared psum tile, allocated lazily at first step
            box = {}

            def step0():
                box["ps"] = psM.tile([P, QCH], F32, tag="psM", name="qkvps")
                emit_half_pass(box["ps"], m, cch, 0)

            def step1():
                emit_half_pass(box["ps"], m, cch, 1)

            def fin():
                if m == 2:
                    emit_v_fin(box["ps"], cch)
                else:
                    emit_kq_fin(box["ps"], q_sb if m == 0 else k_sb, cch)
                done.add((m, cch))

            bg.append((4, step0))
            bg.append((4, step1))
            bg.append((1, fin))

        def run_pass(m, cch):
            ps = psM.tile([P, QCH], F32, tag="psM")
            emit_half_pass(ps, m, cch, 0)
            emit_half_pass(ps, m, cch, 1)
            if m == 2:
                emit_v_fin(ps, cch)
            else:
                emit_kq_fin(ps, q_sb if m == 0 else k_sb, cch)

        # ---------------- scores: 2x2 tile_position pack ----------------
        def emit_scores_kt(jb, b, qc, kt):
            qsl = slice(b * TB + qc * QCH, b * TB + (qc + 1) * QCH)
            k0 = b * TB + kt * P
            s = psA.tile([P, 2, QCH], F32, tag="psA")
            for h in range(2):
                rsl = slice(h * HD, (h + 1) * HD)
                for tp in range(2):
                    csl = slice(k0 + tp * HD, k0 + (tp + 1) * HD)
                    nc.tensor.matmul(
                        s[tp * HD : (tp + 1) * HD, h, :],
                        lhsT=k_sb[rsl, csl],
                        rhs=q_sb[rsl, qsl],
                        start=True,
                        stop=True,
                        tile_position=(h * HD, tp * HD),
                    )
            nc.scalar.activation(out=pT[jb][:, kt, :, :], in_=s[:], func=AFT.Exp)

        # ---------------- y = P @ V, normalize, project ------------------
        def queue_yproj(jb, b, qc):
            box = {}

            def pv_step(kt):
                def fn():
                    if kt == 0:
                        box["y2"] = psPV.tile([P, 2, QCH], F32, tag="psPV", name="y2")
                    y2 = box["y2"]
                    for h in range(2):
                        nc.tensor.matmul(
                            y2[0 : HD + 1, h, :],
                            lhsT=v_aug[:, b * NKT + kt, h, 0 : HD + 1],
                            rhs=pT[jb][:, kt, h, :],
                            start=(kt == 0),
                            stop=(kt == NKT - 1),
                        )
                return fn

            def norm():
                y2 = box["y2"]
                rec = recp.tile([1, 2, QCH], F32, tag="rec")
                nc.vector.reciprocal(out=rec[:], in_=y2[HD : HD + 1, :, :])
                rf = rfp.tile([P, 2, QCH], F32, tag="rf")
                nc.gpsimd.partition_broadcast(out_ap=rf[:], in_ap=rec[:])
                yb = ybp.tile([P, QCH], BF16, tag="yb")
                box["yb"] = yb
                nc.vector.tensor_mul(
                    out=yb[0:HD, :], in0=y2[0:HD, 0, :], in1=rf[0:HD, 0, :]
                )
                nc.vector.tensor_mul(
                    out=yb[HD:P, :], in0=y2[0:HD, 1, :], in1=rf[HD:P, 1, :]
                )

            def proj_tt(tt):
                def fn():
                    # psM rotation is FIFO-safe: qkv chain steps are
                    # adjacent in the deque, so no foreign alloc can land
                    # between a chain's start and its evacuating copy.
                    yb = box["yb"]
                    row0 = b * TB + qc * QCH + tt * P
                    ob = osb.tile([P, C], F32, tag="osb")
                    for ncol in range(2):
                        po = psM.tile([P, QCH], F32, tag="psM")
                        nc.tensor.matmul(
                            po[:],
                            lhsT=yb[:, tt * P : (tt + 1) * P],
                            rhs=wp_sb[:, ncol * QCH : (ncol + 1) * QCH],
                            start=True,
                            stop=True,
                        )
                        nc.vector.tensor_copy(
                            out=ob[:, ncol * QCH : (ncol + 1) * QCH], in_=po[:]
                        )
                    nc.sync.dma_start(out=out[row0 : row0 + P, :], in_=ob[:])
                return fn

            for kt in range(NKT):
                bg.append((2, pv_step(kt)))
            bg.append((1, norm))
            for tt in range(QCH // P):
                bg.append((3, proj_tt(tt)))

        # ---------------- emission schedule ----------------
        # lead-in: k then q for chunk 0 so the exp stream starts ASAP
        run_pass(1, 0)
        run_pass(0, 0)
        done.add((1, 0))
        done.add((0, 0))
        # background: rest of b0's k (needed by scores(b0) kt>=4), then its
        # q chunks, v chunks, then all of b1's qkv
        for cch in (1, 2, 3):
            queue_pass(1, cch)
        for cch in (1, 2, 3):
            queue_pass(0, cch)
        for cch in (0, 1, 2, 3):
            queue_pass(2, cch)
        for cch in (4, 5, 6, 7):
            queue_pass(1, cch)
        for cch in (4, 5, 6, 7):
            queue_pass(0, cch)
        for cch in (4, 5, 6, 7):
            queue_pass(2, cch)

        chunks = [(b, qc) for b in range(NB) for qc in range(NQC)]
        for j, (b, qc) in enumerate(chunks):
            jb = j % 2
            need(0, j)
            if True:  # BISECT: no interleaving
                while bg:
                    _c, _f = bg.popleft()
                    _f()
            for kt in range(NKT):
                need(1, b * NQC + kt // 4)
                emit_scores_kt(jb, b, qc, kt)
                pump(4.4)
            queue_yproj(jb, b, qc)
        pump(10**9)
    return nc


def _prepare_in_maps(x, w_attn, w_proj):
    bf16 = ml_dtypes.bfloat16
    x = np.asarray(x, dtype=np.float32)
    w_attn = np.asarray(w_attn, dtype=np.float32)
    w_proj = np.asarray(w_proj, dtype=np.float32)

    xT = np.ascontiguousarray(x.reshape(NT, C).T.astype(bf16))  # [C, NT]
    in_maps = []
    for c in range(N_CORES):
        h0, h1 = 2 * c, 2 * c + 1
        cols = []
        for h in (h0, h1):  # q columns, pre-scaled by softmax 1/sqrt(64)
            cols.append(w_attn[:, h * HD : (h + 1) * HD] * 0.125)
        for h in (h0, h1):  # k columns
            cols.append(w_attn[:, C + h * HD : C + (h + 1) * HD])
        for h in (h0, h1):  # v columns
            cols.append(w_attn[:, 2 * C + h * HD : 2 * C + (h + 1) * HD])
        wqkv_c = np.ascontiguousarray(np.concatenate(cols, axis=1).astype(bf16))
        wproj_c = np.ascontiguousarray(
            np.stack(
                [
                    w_proj[h0 * HD : (h0 + 1) * HD, :],
                    w_proj[h1 * HD : (h1 + 1) * HD, :],
                ]
            ).astype(bf16)
        )  # [2, 64, C]
        in_maps.append({"xT": xT, "wqkv": wqkv_c, "wproj": wproj_c})
    return in_maps


_CACHED_NC = None


def _get_nc():
    global _CACHED_NC
    if _CACHED_NC is None:
        _CACHED_NC = _build_program(bacc.Bacc())
        _CACHED_NC.finalize()
    return _CACHED_NC


def run(x, w_attn, w_proj, trace=False):
    """Returns (output [B, TB, C] float32, BassKernelResults)."""
    in_maps = _prepare_in_maps(x, w_attn, w_proj)
    nc = _get_nc()
    res = run_bass_kernel_spmd(nc, in_maps, core_ids=list(range(N_CORES)), trace=trace)
    acc = np.zeros((NT, C), dtype=np.float64)
    for r in res.results:
        acc += r["out"].astype(np.float64)
    return acc.astype(np.float32).reshape(NB, TB, C), res


def kernel(x, w_attn, w_proj):
    out, _ = run(x, w_attn, w_proj, trace=False)
    return out
